# revision 27
# baseline (speedup 1.0000x reference)
"""Trainium2 Bass kernel for a batch-of-trees BinaryTreeLSTM (fp16 rewrite).

Contract: kernel(**inputs) takes the FULL inputs (B=128 trees, 1023-node
complete binary tree, dim 300) and returns the FULL output (root_c, root_h),
each [128, 300] float32.

Strategy
--------
- Data-parallel over trees: 16 trees per NeuronCore x 8 cores, no collectives.
- fp16 everywhere: GEMM operands (weights, x, h), gates, c/h states.  fp32
  PSUM accumulate + fp32 ACT internals keep the root error ~1.3e-3, well
  under the 2e-2 gate.  fp16 runs 1 cycle/row on the PE at ANY free size
  (fp32r pays 4x below 256 cols), halves LDWEIGHTS, DMA and SBUF vs fp32r.
- M-repacked gate units: matmul cost is (#units x #K-chunks) x N, independent
  of unit row-width, so the 1500 recurrent gate rows (i,o,u,fL,fR x 300) are
  packed into 13 units (vs 15 naive; 44-row gate tails sit at partition
  offsets 0/64 to satisfy the SBUF base-partition rule) and the 900 leaf gate
  rows into 8 units.
- Bias enters via a ones-row at partition 44 of the packed K-chunk 2 (which
  carries child-h features 256:300 of left/right at partitions 0:44/64:108),
  so ACT applies pure sigmoid/tanh and unit pairs merge into wide ACTs.
- All state is SBUF-resident (fp16 makes it fit); states deinterleaved
  ([even | odd] nodes) per feature chunk: h01/c01 [128, 2R], h2p/c2p
  [128, R/2] with chunk 2 packed into partition bands.
- Phase A software-pipelines leaf blocks against the previous block's level-1
  GEMM so the PE never waits on the leaf elementwise chain.
- DMAs are batched (single wleaf/wrec transfers, 2-DMA x blocks, pad DMAs
  deferred past block 0) so the first matmul issues ~13us after start.
"""

import os
import sys

for _p in ("/opt/trn_rl_repo",):
    if os.path.isdir(_p) and _p not in sys.path:
        sys.path.insert(0, _p)

import numpy as np
from contextlib import ExitStack

import concourse.bass as bass
import concourse.tile as tile
from concourse import bacc, mybir
from concourse.bass_utils import run_bass_kernel_spmd

# ---------------------------------------------------------------- constants
N_CORES = 8
B = 128
B_LOC = B // N_CORES          # 16 trees per core
N_LEAVES = 512
MEM = 300
XCOLS = N_LEAVES * B_LOC      # 8192 leaf columns per core
LB = 1024                     # leaf-block columns (64 leaves)
NF = 512                      # max moving free dim
R_LVL = {l: XCOLS >> l for l in range(1, 10)}   # level l column count

F16 = mybir.dt.float16
F32 = mybir.dt.float32
AF = mybir.ActivationFunctionType
SIG = AF.Sigmoid
TANH = AF.Tanh
MUL = mybir.AluOpType.mult
ADD = mybir.AluOpType.add
F8 = mybir.dt.float8e4
S_W = 512.0      # fp8 f-gate weight scale
S_H = 16.0       # fp8 staged-h scale
F_UNITS = (4, 5, 6, 7)   # fL0, fL1, fR0, fR1 run as fp8 DoubleRow

# (SBUF compute APs must start at partition 0 or 64, so tail gates sit at
# those offsets; rows 44:64 / 108:128 of tail units carry zero weights.)
LEAF_SLOTS = {
    0: [(0, 128, 0)], 1: [(0, 128, 128)],
    2: [(0, 128, 300)], 3: [(0, 128, 428)],
    4: [(0, 128, 600)], 5: [(0, 128, 728)],
    6: [(0, 44, 256), (64, 108, 556)],   # i2 @0 | o2 @64
    7: [(0, 44, 856)],                   # u2 @0
}
# Recurrent M-units (13 units over Wcat cols
# [i 0:300 | o 300:600 | u 600:900 | fL 900:1200 | fR 1200:1500]):
REC_SLOTS = {
    0: [(0, 128, 0)], 1: [(0, 128, 128)],
    2: [(0, 128, 300)], 3: [(0, 128, 428)],
    4: [(0, 128, 900)], 5: [(0, 128, 1028)],     # fL
    6: [(0, 128, 1200)], 7: [(0, 128, 1328)],    # fR
    8: [(0, 128, 600)], 9: [(0, 128, 728)],      # u
    10: [(0, 44, 256), (64, 108, 556)],          # i2 @0 | o2 @64
    11: [(0, 44, 856), (64, 108, 1156)],         # u2 @0 | fL2 @64
    12: [(0, 44, 1456)],                         # fR2 @0
}


# ---------------------------------------------------------------- host packing
def _pack_weights(Wfioux, b_fioux, Wiouh, Wfh):
    f4 = np.float32
    Wiou = np.asarray(Wfioux[:, 300:1200], f4)            # [300, 900]
    bleaf = np.asarray(b_fioux[300:1200], f4)             # [900]
    wleaf = np.zeros((3, 128, 8 * 128), f4)
    kch_l = [(0, 128), (128, 256), (256, 300)]
    for kc, (ra, rb) in enumerate(kch_l):
        for m, slots in LEAF_SLOTS.items():
            for (r0, r1, c0) in slots:
                wleaf[kc, 0: rb - ra, 128 * m + r0: 128 * m + r1] = \
                    Wiou[ra:rb, c0: c0 + (r1 - r0)]
    # bias via ones-row at partition 44 of K-chunk 2
    for m, slots in LEAF_SLOTS.items():
        for (r0, r1, c0) in slots:
            wleaf[2, 44, 128 * m + r0: 128 * m + r1] = bleaf[c0: c0 + (r1 - r0)]

    Wcat = np.concatenate([Wiouh, Wfh], axis=1).astype(f4)  # [600, 1500]
    bf = np.asarray(b_fioux[0:300], f4)
    bias_cat = np.concatenate(
        [b_fioux[300:600], b_fioux[600:900], b_fioux[900:1200], bf, bf]
    ).astype(f4)
    wrec = np.zeros((5, 128, 13 * 128), f4)
    # K-chunks: 0: hL[0:128], 1: hL[128:256], 2: packed hL[256:300]@0:44 +
    # ones@44 + hR[256:300]@64:108, 3: hR[0:128], 4: hR[128:256]
    kch_r = [(0, 128), (128, 256), None, (300, 428), (428, 556)]
    for kc, span in enumerate(kch_r):
        if span is None:
            continue
        ra, rb = span
        for m, slots in REC_SLOTS.items():
            for (r0, r1, c0) in slots:
                wrec[kc, 0: rb - ra, 128 * m + r0: 128 * m + r1] = \
                    Wcat[ra:rb, c0: c0 + (r1 - r0)]
    for m, slots in REC_SLOTS.items():
        for (r0, r1, c0) in slots:
            wrec[2, 0:44, 128 * m + r0: 128 * m + r1] = \
                Wcat[256:300, c0: c0 + (r1 - r0)]
            wrec[2, 44, 128 * m + r0: 128 * m + r1] = bias_cat[c0: c0 + (r1 - r0)]
            wrec[2, 64:108, 128 * m + r0: 128 * m + r1] = \
                Wcat[556:600, c0: c0 + (r1 - r0)]
    wleaf_f = np.ascontiguousarray(
        wleaf.transpose(1, 0, 2).reshape(128, 3 * 1024))
    wrec_f = np.ascontiguousarray(
        wrec.transpose(1, 0, 2).reshape(128, 5 * 1664))
    # fp8 copies of the main f-gate units, scaled by S_W.  Layout:
    # [0:1024] kc0/kc1 pairs (4 units x [two x 128]), [1024:2048] kc3/kc4,
    # [2048:2560] kc2 (4 x 128).
    import ml_dtypes
    w8 = np.zeros((128, 2560), np.float32)
    for fu, u in enumerate(F_UNITS):
        c0 = REC_SLOTS[u][0][2]
        for two, ra in ((0, 0), (1, 128)):
            w8[:, fu * 256 + two * 128: fu * 256 + two * 128 + 128] =                 Wcat[ra: ra + 128, c0: c0 + 128]
        for two, ra in ((0, 300), (1, 428)):
            w8[:, 1024 + fu * 256 + two * 128: 1024 + fu * 256 + two * 128 + 128] =                 Wcat[ra: ra + 128, c0: c0 + 128]
        w8[0:44, 2048 + fu * 128: 2048 + fu * 128 + 128] =             Wcat[256:300, c0: c0 + 128]
        w8[44, 2048 + fu * 128: 2048 + fu * 128 + 128] = bias_cat[c0: c0 + 128]
        w8[64:108, 2048 + fu * 128: 2048 + fu * 128 + 128] =             Wcat[556:600, c0: c0 + 128]
    wrec8 = (w8 * S_W).astype(ml_dtypes.float8_e4m3fn)
    return (wleaf_f.astype(np.float16), wrec_f.astype(np.float16), wrec8)


def _check_topology(left_idx, right_idx, leaf_mask):
    li = np.asarray(left_idx); ri = np.asarray(right_idx)
    prev = np.arange(N_LEAVES); nid = N_LEAVES
    ok = bool((np.asarray(leaf_mask)[:N_LEAVES] == 1).all())
    ok &= bool((np.asarray(leaf_mask)[N_LEAVES:] == 0).all())
    while len(prev) > 1:
        cur = []
        for k in range(0, len(prev), 2):
            ok &= bool(li[nid] == prev[k]) and bool(ri[nid] == prev[k + 1])
            cur.append(nid); nid += 1
        prev = np.asarray(cur)
    return ok


# ---------------------------------------------------------------- bass program
def _ev_od(ap, b=B_LOC):
    """Block-dense [p, X] (node-major, X = m*2*b) -> (even, odd) [p, m, b]."""
    r = ap.rearrange("p (m two b) -> p m two b", two=2, b=b)
    return r[:, :, 0, :], r[:, :, 1, :]


def _mb(ap, b=B_LOC):
    return ap.rearrange("p (m b) -> p m b", b=b)


def build_program():
    nc = bacc.Bacc("TRN2", target_bir_lowering=False, debug=False)

    xt_d = nc.dram_tensor("xt", [128, 2, XCOLS], F16, kind="ExternalInput").ap()
    x2t_d = nc.dram_tensor("x2t", [44, XCOLS], F16, kind="ExternalInput").ap()
    wleaf_d = nc.dram_tensor("wleaf", [128, 3 * 8 * 128], F16,
                             kind="ExternalInput").ap()
    wrec_d = nc.dram_tensor("wrec", [128, 5 * 13 * 128], F16,
                            kind="ExternalInput").ap()
    wrec8_d = nc.dram_tensor("wrec8", [128, 2560], F8,
                             kind="ExternalInput").ap()
    cons_d = nc.dram_tensor("cons", [84, 2 * LB], F16, kind="ExternalInput").ap()
    out_d = nc.dram_tensor("out", [128, 6 * B_LOC], F16,
                       kind="ExternalOutput").ap()

    with ExitStack() as ctx:
        tc = ctx.enter_context(tile.TileContext(nc))
        _build(ctx, tc, xt_d, x2t_d, wleaf_d, wrec_d, wrec8_d, cons_d, out_d)

    nc.compile()
    return nc


def _build(ctx, tc, xt_d, x2t_d, wleaf_d, wrec_d, wrec8_d, cons_d, out_d):
    nc = tc.nc

    wpool = ctx.enter_context(tc.tile_pool(name="wpool", bufs=1))
    state_pool = ctx.enter_context(tc.tile_pool(name="state", bufs=1))

    # ---- weights resident in SBUF (leaf weights first: needed immediately)
    wleaf_t = wpool.tile([128, 3 * 8 * 128], F16, name="wleaf")
    nc.sync.dma_start(wleaf_t[:], wleaf_d[:])
    wleaf_sb = [wleaf_t[:, k * 1024: (k + 1) * 1024] for k in range(3)]
    wrec_t = wpool.tile([128, 5 * 13 * 128], F16, name="wrec")
    wrec_sb = [wrec_t[:, k * 1664: (k + 1) * 1664] for k in range(5)]
    wrec8_t = wpool.tile([128, 2560], F8, name="wrec8")

    # ---- persistent SBUF state for levels 1..8
    # h01/c01 [128, 2R]: chunks 0,1, each [E | O].  h2p [128, R/2]: chunk-2
    # packed E@0:44 / O@64:108 (+ ones row 44 for the GEMM bias).
    # c2p [128, R/2]: chunk-2 E@64:108 / O@0:44 (so fL2@64*CL2 and fR2@0*CR2
    # have matching input base partitions).
    ST = {}
    for lvl in range(1, 9):
        R = R_LVL[lvl]
        h01 = state_pool.tile([128, 2 * R], F16, name=f"h01_{lvl}")
        h2p = state_pool.tile([128, R // 2], F16, name=f"h2p_{lvl}")
        c01 = state_pool.tile([128, 2 * R], F16, name=f"c01_{lvl}")
        c2p = state_pool.tile([128, R // 2], F16, name=f"c2p_{lvl}")
        ST[lvl] = dict(h01=h01, h2p=h2p, c01=c01, c2p=c2p, R=R)

    # persistent double-buffered leaf tiles that carry constant rows
    x2_t = [state_pool.tile([128, LB], F16, name="x2_0")]
    x2_t.append(x2_t[0])
    lh2p_t = []
    for i in range(2):
        lh2p_t.append(state_pool.tile([128, LB // 2], F16, name=f"lh2p_{i}"))

    def _pad_dmas():
        """Pad/ones constants for tiles not needed in the first block; emitted
        after the block-0 x DMAs so compute starts immediately."""
        for i in range(2):
            t = lh2p_t[i]
            nc.sync.dma_start(t[44:64, :], cons_d[0:20, : LB // 2])
            nc.sync.dma_start(t[108:128, :], cons_d[1:21, : LB // 2])
        for lvl in range(1, 9):
            R = R_LVL[lvl]
            h2p = ST[lvl]["h2p"]
            nc.sync.dma_start(h2p[44:64, :], cons_d[0:20, : R // 2])
            nc.sync.dma_start(h2p[108:128, :], cons_d[1:21, : R // 2])

    # ---- pools
    xpool = ctx.enter_context(tc.tile_pool(name="xpool", bufs=2))
    glpool = ctx.enter_context(tc.tile_pool(name="gl", bufs=3))
    lpool = ctx.enter_context(tc.tile_pool(name="lpool", bufs=2))
    gpool = ctx.enter_context(tc.tile_pool(name="g", bufs=2))
    pspool = ctx.enter_context(tc.tile_pool(name="ps", bufs=4, space="PSUM"))
    tmp1 = ctx.enter_context(tc.tile_pool(name="tmp1", bufs=1))
    tmp2 = ctx.enter_context(tc.tile_pool(name="tmp2", bufs=2))
    tmp3 = ctx.enter_context(tc.tile_pool(name="tmp3", bufs=1))
    opool = ctx.enter_context(tc.tile_pool(name="o", bufs=1))
    s8pool = ctx.enter_context(tc.tile_pool(name="s8", bufs=1))

    # ================================================================ helpers
    def leaf_gemm(xk, s, Gl):
        """Leaf gates for sub-chunk s (512 cols): 4 psum pairs, 5 ACTs."""
        n0 = s * NF
        for pi in range(4):
            ua, ub = 2 * pi, 2 * pi + 1
            ps = pspool.tile([128, 2 * NF], F32, tag="ps", name=f"psl{pi}")
            for j, u in enumerate((ua, ub)):
                rows = (128, 128, 128, 128, 128, 128, 108, 44)[u]
                off = j * NF
                for kc in range(3):
                    nc.tensor.matmul(
                        ps[0:rows, off: off + NF],
                        wleaf_sb[kc][:, 128 * u: 128 * u + rows],
                        xk[kc][:, n0: n0 + NF],
                        start=(kc == 0), stop=(kc == 2))
            if pi < 3:
                func = SIG if pi < 2 else TANH
                nc.scalar.activation(Gl[:, 2 * pi * NF: (2 * pi + 2) * NF],
                                     ps[:, :], func)
            else:
                nc.scalar.activation(Gl[0:108, 6 * NF: 7 * NF],
                                     ps[0:108, 0:NF], SIG)
                nc.scalar.activation(Gl[0:44, 7 * NF: 8 * NF],
                                     ps[0:44, NF: NF + NF], TANH)

    def rec_gemm(rhs_k, PB, G, fp8=True):
        """Recurrent gates for one block of PB cols.  The i/o/u units and the
        tail units run fp16 (5 K-chunk matmuls each); the 4 main f units run
        fp8 DoubleRow (2 DR pairs + 1 plain fp8 K-chunk) against an
        S_H-scaled fp8 copy of the child h staged on GpSimd.  The f-gate ACTs
        descale by 1/(S_W*S_H)."""
        # stage an S_H-scaled fp8 rhs copy (5 chunks side by side) on DVE
        if fp8:
            st8 = s8pool.tile([128, 5 * NF], F8, tag="st8", name="st8")
            for kc in range(5):
                nc.vector.tensor_scalar_mul(st8[:, kc * PB: (kc + 1) * PB],
                                            rhs_k[kc], S_H)
            dr = st8[:, 0: 5 * PB].rearrange("p (kc n) -> p kc n", kc=5)

        # iou + tail pairs (fp16), then f pairs (fp8)
        for pi, units in enumerate(((0, 1), (2, 3), (8, 9), (10, 11), (12,),
                                    (4, 5), (6, 7))):
            ps = pspool.tile([128, 2 * NF], F32, tag="ps", name=f"psr{pi}")
            for j, u in enumerate(units):
                off = j * PB
                if u in F_UNITS and fp8:
                    fu = u - 4
                    w01 = wrec8_t[:, fu * 256: (fu + 1) * 256].rearrange(
                        "p (two m) -> p two m", two=2)
                    w34 = wrec8_t[:, 1024 + fu * 256: 1024 + (fu + 1) * 256]                         .rearrange("p (two m) -> p two m", two=2)
                    w2 = wrec8_t[:, 2048 + fu * 128: 2048 + (fu + 1) * 128]
                    nc.tensor.matmul(ps[:, off: off + PB], w01, dr[:, 0:2, :],
                                     start=True, stop=False,
                                     perf_mode=mybir.MatmulPerfMode.DoubleRow)
                    nc.tensor.matmul(ps[:, off: off + PB], w34, dr[:, 3:5, :],
                                     start=False, stop=False,
                                     perf_mode=mybir.MatmulPerfMode.DoubleRow)
                    nc.tensor.matmul(ps[:, off: off + PB], w2,
                                     st8[:, 2 * PB: 3 * PB],
                                     start=False, stop=True)
                    continue
                rows = 108 if u in (10, 11) else (44 if u == 12 else 128)
                for kc in range(5):
                    nc.tensor.matmul(
                        ps[0:rows, off: off + PB],
                        wrec_sb[kc][:, 128 * u: 128 * u + rows],
                        rhs_k[kc],
                        start=(kc == 0), stop=(kc == 4))
            if pi < 2:
                nc.scalar.activation(G[:, 2 * pi * PB: (2 * pi + 2) * PB],
                                     ps[:, 0: 2 * PB], SIG)
            elif pi == 2:
                nc.scalar.activation(G[:, 8 * PB: 10 * PB], ps[:, 0: 2 * PB],
                                     TANH)
            elif pi == 3:
                # T10 = [i2@0 | o2@64] all sigmoid; T11 = [u2@0 | fL2@64]
                nc.scalar.activation(G[0:108, 10 * PB: 11 * PB],
                                     ps[0:108, 0:PB], SIG)
                nc.scalar.activation(G[0:44, 11 * PB: 12 * PB],
                                     ps[0:44, PB: 2 * PB], TANH)
                nc.scalar.activation(G[64:108, 11 * PB: 12 * PB],
                                     ps[64:108, PB: 2 * PB], SIG)
            elif pi == 4:
                nc.scalar.activation(G[0:44, 12 * PB: 13 * PB],
                                     ps[0:44, 0:PB], SIG)
            elif pi == 5:
                nc.scalar.activation(G[:, 4 * PB: 6 * PB], ps[:, 0: 2 * PB],
                                     SIG,
                                     scale=1.0 / (S_W * S_H) if fp8 else 1.0)
            else:
                nc.scalar.activation(G[:, 6 * PB: 8 * PB], ps[:, 0: 2 * PB],
                                     SIG,
                                     scale=1.0 / (S_W * S_H) if fp8 else 1.0)

    def st_sl(t, R, ch, eo, q0, w):
        off = ch * R + eo * (R // 2) + q0
        return t[:, off: off + w]

    def rec_ew(G, PB, CL, CR, dst, p0):
        """Elementwise for a recurrent block. G gates [128, 13*PB].
        CL/CR: (c0, c1, c2) child-c dense APs [., PB] (c2: 44 rows at base
        64 (CL) / 0 (CR) to match fL2/fR2 partition bases).
        dst: ST[lvl] dict, or ('root', oc01, oc2, oh01, oh2) for level 9."""
        N = PB
        gi = [G[:, 0:N], G[:, N: 2 * N], G[0:44, 10 * N: 11 * N]]
        go = [G[:, 2 * N: 3 * N], G[:, 3 * N: 4 * N], G[64:108, 10 * N: 11 * N]]
        gfL = [G[:, 4 * N: 5 * N], G[:, 5 * N: 6 * N], G[64:108, 11 * N: 12 * N]]
        gfR = [G[:, 6 * N: 7 * N], G[:, 7 * N: 8 * N], G[0:44, 12 * N: 13 * N]]
        gu = [G[:, 8 * N: 9 * N], G[:, 9 * N: 10 * N], G[0:44, 11 * N: 12 * N]]

        t1 = tmp1.tile([128, 2 * NF], F16, tag="t1", name="t1")
        t2 = tmp1.tile([128, 2 * NF], F16, tag="t2", name="t2")
        fc = tmp1.tile([128, 2 * NF], F16, tag="fc", name="fc")
        iu = tmp1.tile([128, 2 * NF], F16, tag="iu", name="iu")
        t1_2 = tmp1.tile([44, NF], F16, tag="t1_2", name="t1_2")
        t2_2 = tmp1.tile([44, NF], F16, tag="t2_2", name="t2_2")
        fc2 = tmp1.tile([44, NF], F16, tag="fc2", name="fc2")
        iu2 = tmp1.tile([44, NF], F16, tag="iu2", name="iu2")

        for ch in range(2):
            nc.vector.tensor_tensor(t1[:, ch * N: (ch + 1) * N], gfL[ch],
                                    CL[ch], MUL)
            nc.vector.tensor_tensor(t2[:, ch * N: (ch + 1) * N], gfR[ch],
                                    CR[ch], MUL)
        nc.vector.tensor_tensor(t1_2[:, :N], gfL[2], CL[2], MUL)
        nc.vector.tensor_tensor(t2_2[:, :N], gfR[2], CR[2], MUL)
        nc.vector.tensor_tensor(fc[:, : 2 * N], t1[:, : 2 * N], t2[:, : 2 * N],
                                ADD)
        nc.vector.tensor_tensor(fc2[:, :N], t1_2[:, :N], t2_2[:, :N], ADD)
        nc.vector.tensor_tensor(iu[:, : 2 * N], G[:, 0: 2 * N],
                                G[:, 8 * N: 10 * N], MUL)
        nc.vector.tensor_tensor(iu2[:, :N], gi[2], gu[2], MUL)

        if isinstance(dst, tuple) and dst[0] == "root":
            _, oc01, oc2, oh01, oh2 = dst
            nc.vector.tensor_tensor(oc01[:, : 2 * N], iu[:, : 2 * N],
                                    fc[:, : 2 * N], ADD)
            nc.vector.tensor_tensor(oc2[:, :N], iu2[:, :N], fc2[:, :N], ADD)
            th = tmp3.tile([128, 2 * NF], F16, tag="th", name="th")
            th2 = tmp3.tile([128, NF], F16, tag="th2", name="th2")
            nc.scalar.activation(th[:, : 2 * N], oc01[:, : 2 * N], TANH)
            nc.scalar.activation(th2[64:108, :N], oc2[:, :N], TANH)
            nc.vector.tensor_tensor(oh01[:, : 2 * N], G[:, 2 * N: 4 * N],
                                    th[:, : 2 * N], MUL)
            nc.vector.tensor_tensor(oh2[:, :N], go[2], th2[64:108, :N], MUL)
            return

        st = dst
        R = st["R"]
        q0, hw = p0 // 2, PB // 2
        # c writes (deinterleave into state), then tanh, then h writes
        for ch in range(2):
            iue, iuo = _ev_od(iu[:, ch * N: (ch + 1) * N])
            fce, fco = _ev_od(fc[:, ch * N: (ch + 1) * N])
            nc.vector.tensor_tensor(_mb(st_sl(st["c01"], R, ch, 0, q0, hw)),
                                    iue, fce, ADD)
            nc.vector.tensor_tensor(_mb(st_sl(st["c01"], R, ch, 1, q0, hw)),
                                    iuo, fco, ADD)
        iue, iuo = _ev_od(iu2[:, :N])
        fce, fco = _ev_od(fc2[:, :N])
        nc.vector.tensor_tensor(_mb(st["c2p"][64:108, q0: q0 + hw]), iue, fce,
                                ADD)
        nc.vector.tensor_tensor(_mb(st["c2p"][0:44, q0: q0 + hw]), iuo, fco,
                                ADD)

        # th layout: [ch0E | ch0O | ch1E | ch1O], each hw wide
        th = tmp3.tile([128, 2 * NF], F16, tag="th", name="th")
        th2 = tmp3.tile([128, NF], F16, tag="th2", name="th2")
        c4 = st["c01"].rearrange("p (ch eo q) -> p ch eo q", ch=2, eo=2)
        tho = th[:, : 2 * N].rearrange("p (ch eo q) -> p ch eo q", ch=2, eo=2)
        nc.scalar.activation(tho, c4[:, :, :, q0: q0 + hw], TANH)
        nc.scalar.activation(th2[64:108, 0:hw], st["c2p"][64:108, q0: q0 + hw],
                             TANH)
        nc.scalar.activation(th2[64:108, hw:N], st["c2p"][0:44, q0: q0 + hw],
                             TANH)

        for ch in range(2):
            oe, oo = _ev_od(go[ch])
            nc.vector.tensor_tensor(_mb(st_sl(st["h01"], R, ch, 0, q0, hw)),
                                    oe, _mb(th[:, ch * N: ch * N + hw]), MUL)
            nc.vector.tensor_tensor(
                _mb(st_sl(st["h01"], R, ch, 1, q0, hw)), oo,
                _mb(th[:, ch * N + hw: ch * N + N]), MUL)
        oe, oo = _ev_od(go[2])
        nc.vector.tensor_tensor(_mb(st["h2p"][0:44, q0: q0 + hw]), oe,
                                _mb(th2[64:108, 0:hw]), MUL)
        nc.vector.tensor_tensor(_mb(st["h2p"][64:108, q0: q0 + hw]), oo,
                                _mb(th2[64:108, hw:N]), MUL)

    def leaf_ew(Gl, s, lh01, lh2p, lc01, lc2p):
        """Leaf elementwise for sub-chunk s (512 cols): c = i*u, h = o*tanh(c).
        Writes deinterleaved into the LB-wide block-local leaf tiles."""
        N = NF
        q0, hw = s * (NF // 2), NF // 2
        gi = [Gl[:, 0:N], Gl[:, N: 2 * N], Gl[0:44, 6 * N: 7 * N]]
        go = [Gl[:, 2 * N: 3 * N], Gl[:, 3 * N: 4 * N], Gl[64:108, 6 * N: 7 * N]]
        gu = [Gl[:, 4 * N: 5 * N], Gl[:, 5 * N: 6 * N], Gl[0:44, 7 * N: 8 * N]]

        for ch in range(2):
            ie, io = _ev_od(gi[ch])
            ue, uo = _ev_od(gu[ch])
            nc.vector.tensor_tensor(_mb(st_sl(lc01, LB, ch, 0, q0, hw)), ie, ue,
                                    MUL)
            nc.vector.tensor_tensor(_mb(st_sl(lc01, LB, ch, 1, q0, hw)), io, uo,
                                    MUL)
        i2e, i2o = _ev_od(gi[2])
        u2e, u2o = _ev_od(gu[2])
        nc.vector.tensor_tensor(_mb(lc2p[64:108, q0: q0 + hw]), i2e, u2e, MUL)
        nc.vector.tensor_tensor(_mb(lc2p[0:44, q0: q0 + hw]), i2o, u2o, MUL)

        th = tmp2.tile([128, 2 * NF], F16, tag="thl", name="lth")
        th2 = tmp3.tile([128, NF], F16, tag="th2l", name="lth2")
        c4 = lc01.rearrange("p (ch eo q) -> p ch eo q", ch=2, eo=2)
        tho = th[:, : 2 * N].rearrange("p (ch eo q) -> p ch eo q", ch=2, eo=2)
        nc.scalar.activation(tho, c4[:, :, :, q0: q0 + hw], TANH)
        nc.scalar.activation(th2[64:108, 0:hw], lc2p[64:108, q0: q0 + hw],
                             TANH)
        nc.scalar.activation(th2[64:108, hw:N], lc2p[0:44, q0: q0 + hw], TANH)

        for ch in range(2):
            oe, oo = _ev_od(go[ch])
            nc.vector.tensor_tensor(_mb(st_sl(lh01, LB, ch, 0, q0, hw)), oe,
                                    _mb(th[:, ch * N: ch * N + hw]), MUL)
            nc.vector.tensor_tensor(
                _mb(st_sl(lh01, LB, ch, 1, q0, hw)), oo,
                _mb(th[:, ch * N + hw: ch * N + N]), MUL)
        oe, oo = _ev_od(go[2])
        nc.vector.tensor_tensor(_mb(lh2p[0:44, q0: q0 + hw]), oe,
                                _mb(th2[64:108, 0:hw]), MUL)
        nc.vector.tensor_tensor(_mb(lh2p[64:108, q0: q0 + hw]), oo,
                                _mb(th2[64:108, hw:N]), MUL)

    # ================================================================ phase A
    # leaves + level-1, software-pipelined: L1 GEMM of block b-1 is emitted
    # after the leaf GEMMs of block b so the PE never waits on leaf DVE.
    n_blk = XCOLS // LB                       # 8 blocks
    pend = None                               # (lh01, lh2p, lc01, lc2p, blk)

    def l1_block(lh01, lh2p, lc01, lc2p, blk):
        rhs_k = [st_sl(lh01, LB, 0, 0, 0, NF), st_sl(lh01, LB, 1, 0, 0, NF),
                 lh2p[:, :],
                 st_sl(lh01, LB, 0, 1, 0, NF), st_sl(lh01, LB, 1, 1, 0, NF)]
        G = gpool.tile([128, 13 * NF], F16, tag="G", name="G1")
        rec_gemm(rhs_k, NF, G)
        CL = [st_sl(lc01, LB, 0, 0, 0, NF), st_sl(lc01, LB, 1, 0, 0, NF),
              lc2p[64:108, :]]
        CR = [st_sl(lc01, LB, 0, 1, 0, NF), st_sl(lc01, LB, 1, 1, 0, NF),
              lc2p[0:44, :]]
        rec_ew(G, NF, CL, CR, ST[1], blk * NF)

    for blk in range(n_blk):
        c0 = blk * LB
        x01 = xpool.tile([128, 2 * LB], F16, tag="x01", name="x01")
        x2 = x2_t[blk % 2]
        nc.sync.dma_start(x01.rearrange("p (two b) -> p two b", two=2),
                          xt_d[:, :, c0: c0 + LB])
        nc.sync.dma_start(x2[0:44, :], x2t_d[:, c0: c0 + LB])
        if blk == 0:
            nc.sync.dma_start(x2[44:128, :], cons_d[0:84, :LB])
            nc.sync.dma_start(wrec_t[:], wrec_d[:])
            nc.sync.dma_start(wrec8_t[:], wrec8_d[:])
        elif blk == 1:
            _pad_dmas()
        xk = [x01[:, 0:LB], x01[:, LB: 2 * LB], x2]

        lh01 = lpool.tile([128, 2 * LB], F16, tag="lh01", name="lh01")
        lh2p = lh2p_t[blk % 2]
        lc01 = lpool.tile([128, 2 * LB], F16, tag="lc01", name="lc01")
        lc2p = lpool.tile([128, LB // 2], F16, tag="lc2p", name="lc2p")

        Gls = []
        for s in range(2):
            Gl = glpool.tile([128, 8 * NF], F16, tag="Gl", name="Gl")
            leaf_gemm(xk, s, Gl)
            Gls.append(Gl)
        if pend is not None:
            l1_block(*pend)
        for s in range(2):
            leaf_ew(Gls[s], s, lh01, lh2p, lc01, lc2p)
        pend = (lh01, lh2p, lc01, lc2p, blk)
    l1_block(*pend)

    # ================================================================ phase B
    for lvl in range(2, 10):
        R = R_LVL[lvl]
        Rp = R_LVL[lvl - 1]
        PB = min(NF, R)
        prev = ST[lvl - 1]
        for p0 in range(0, R, PB):
            rhs_k = [st_sl(prev["h01"], Rp, 0, 0, p0, PB),
                     st_sl(prev["h01"], Rp, 1, 0, p0, PB),
                     prev["h2p"][:, p0: p0 + PB],
                     st_sl(prev["h01"], Rp, 0, 1, p0, PB),
                     st_sl(prev["h01"], Rp, 1, 1, p0, PB)]
            G = gpool.tile([128, 13 * NF], F16, tag="G", name=f"G{lvl}")
            rec_gemm(rhs_k, PB, G[:, : 13 * PB], fp8=(PB == NF))
            CL = [st_sl(prev["c01"], Rp, 0, 0, p0, PB),
                  st_sl(prev["c01"], Rp, 1, 0, p0, PB),
                  prev["c2p"][64:108, p0: p0 + PB]]
            CR = [st_sl(prev["c01"], Rp, 0, 1, p0, PB),
                  st_sl(prev["c01"], Rp, 1, 1, p0, PB),
                  prev["c2p"][0:44, p0: p0 + PB]]
            if lvl < 9:
                rec_ew(G[:, : 13 * PB], PB, CL, CR, ST[lvl], p0)
            else:
                ot = opool.tile([128, 6 * B_LOC], F16, name="ot")
                nc.sync.dma_start(ot[44:128, 2 * B_LOC: 3 * B_LOC],
                                  cons_d[0:84, B_LOC: 2 * B_LOC])
                nc.sync.dma_start(ot[44:128, 5 * B_LOC: 6 * B_LOC],
                                  cons_d[0:84, B_LOC: 2 * B_LOC])
                rec_ew(G[:, : 13 * PB], PB, CL, CR,
                       ("root", ot[:, 0: 2 * B_LOC], ot[0:44, 2 * B_LOC: 3 * B_LOC],
                        ot[:, 3 * B_LOC: 5 * B_LOC], ot[0:44, 5 * B_LOC: 6 * B_LOC]),
                       p0)
                nc.sync.dma_start(out_d[:, :], ot[:, :])


# ---------------------------------------------------------------- runner
_CACHE = {}


def _get_program():
    if "nc" not in _CACHE:
        _CACHE["nc"] = build_program()
    return _CACHE["nc"]


def _host_inputs(inputs, Wfioux, b_fioux, Wiouh, Wfh):
    wleaf, wrec, wrec8 = _pack_weights(
        np.asarray(Wfioux, np.float32), np.asarray(b_fioux, np.float32),
        np.asarray(Wiouh, np.float32), np.asarray(Wfh, np.float32))
    cons = np.zeros((84, 2 * LB), np.float16)
    cons[0, :] = 1.0
    in_maps = []
    for core in range(N_CORES):
        x = np.asarray(inputs[core * B_LOC:(core + 1) * B_LOC, :N_LEAVES, :],
                       np.float32)
        xt_full = x.transpose(2, 1, 0).reshape(MEM, XCOLS).astype(np.float16)
        xt = np.ascontiguousarray(xt_full[0:256].reshape(2, 128, XCOLS)
                                  .transpose(1, 0, 2))
        x2t = np.ascontiguousarray(xt_full[256:300])
        in_maps.append({"xt": xt, "x2t": x2t, "wleaf": wleaf, "wrec": wrec,
                        "wrec8": wrec8, "cons": cons})
    return in_maps


def kernel(inputs, Wfioux, b_fioux, Wiouh, Wfh, left_idx, right_idx, leaf_mask,
           _trace=False, _trace_dir=None):
    inputs = np.asarray(inputs, np.float32)
    assert _check_topology(left_idx, right_idx, leaf_mask), \
        "tree topology does not match the expected complete binary tree"

    in_maps = _host_inputs(inputs, Wfioux, b_fioux, Wiouh, Wfh)
    nc = _get_program()
    res = run_bass_kernel_spmd(nc, in_maps, list(range(N_CORES)),
                               trace=_trace, tmpdir=_trace_dir)

    root_c = np.zeros((B, MEM), np.float32)
    root_h = np.zeros((B, MEM), np.float32)
    for core in range(N_CORES):
        out = np.asarray(res.results[core]["out"], np.float32)  # [128, 96]
        sl = slice(core * B_LOC, (core + 1) * B_LOC)
        root_c[sl, 0:128] = out[:, 0:16].T
        root_c[sl, 128:256] = out[:, 16:32].T
        root_c[sl, 256:300] = out[0:44, 32:48].T
        root_h[sl, 0:128] = out[:, 48:64].T
        root_h[sl, 128:256] = out[:, 64:80].T
        root_h[sl, 256:300] = out[0:44, 80:96].T
    _CACHE["last_results"] = res
    return root_c, root_h


# revision 29
# speedup vs baseline: 1.0083x; 1.0083x over previous
"""Trainium2 Bass kernel for a batch-of-trees BinaryTreeLSTM (fp16 rewrite).

Contract: kernel(**inputs) takes the FULL inputs (B=128 trees, 1023-node
complete binary tree, dim 300) and returns the FULL output (root_c, root_h),
each [128, 300] float32.

Strategy
--------
- Data-parallel over trees: 16 trees per NeuronCore x 8 cores, no collectives.
- fp16 everywhere: GEMM operands (weights, x, h), gates, c/h states.  fp32
  PSUM accumulate + fp32 ACT internals keep the root error ~1.3e-3, well
  under the 2e-2 gate.  fp16 runs 1 cycle/row on the PE at ANY free size
  (fp32r pays 4x below 256 cols), halves LDWEIGHTS, DMA and SBUF vs fp32r.
- M-repacked gate units: matmul cost is (#units x #K-chunks) x N, independent
  of unit row-width, so the 1500 recurrent gate rows (i,o,u,fL,fR x 300) are
  packed into 13 units (vs 15 naive; 44-row gate tails sit at partition
  offsets 0/64 to satisfy the SBUF base-partition rule) and the 900 leaf gate
  rows into 8 units.
- Bias enters via a ones-row at partition 44 of the packed K-chunk 2 (which
  carries child-h features 256:300 of left/right at partitions 0:44/64:108),
  so ACT applies pure sigmoid/tanh and unit pairs merge into wide ACTs.
- All state is SBUF-resident (fp16 makes it fit); states deinterleaved
  ([even | odd] nodes) per feature chunk: h01/c01 [128, 2R], h2p/c2p
  [128, R/2] with chunk 2 packed into partition bands.
- Phase A software-pipelines leaf blocks against the previous block's level-1
  GEMM so the PE never waits on the leaf elementwise chain.
- DMAs are batched (single wleaf/wrec transfers, 2-DMA x blocks, pad DMAs
  deferred past block 0) so the first matmul issues ~13us after start.
"""

import os
import sys

for _p in ("/opt/trn_rl_repo",):
    if os.path.isdir(_p) and _p not in sys.path:
        sys.path.insert(0, _p)

import numpy as np
from contextlib import ExitStack

import concourse.bass as bass
import concourse.tile as tile
from concourse import bacc, mybir
from concourse.bass_utils import run_bass_kernel_spmd

# ---------------------------------------------------------------- constants
N_CORES = 8
B = 128
B_LOC = B // N_CORES          # 16 trees per core
N_LEAVES = 512
MEM = 300
XCOLS = N_LEAVES * B_LOC      # 8192 leaf columns per core
LB = 1024                     # leaf-block columns (64 leaves)
NF = 512                      # max moving free dim
R_LVL = {l: XCOLS >> l for l in range(1, 10)}   # level l column count

F16 = mybir.dt.float16
F32 = mybir.dt.float32
AF = mybir.ActivationFunctionType
SIG = AF.Sigmoid
TANH = AF.Tanh
MUL = mybir.AluOpType.mult
ADD = mybir.AluOpType.add
F8 = mybir.dt.float8e4
S_W = 512.0      # fp8 f-gate weight scale
S_H = 16.0       # fp8 staged-h scale
F_UNITS = (4, 5, 6, 7)   # fL0, fL1, fR0, fR1 run as fp8 DoubleRow

# (SBUF compute APs must start at partition 0 or 64, so tail gates sit at
# those offsets; rows 44:64 / 108:128 of tail units carry zero weights.)
LEAF_SLOTS = {
    0: [(0, 128, 0)], 1: [(0, 128, 128)],
    2: [(0, 128, 300)], 3: [(0, 128, 428)],
    4: [(0, 128, 600)], 5: [(0, 128, 728)],
    6: [(0, 44, 256), (64, 108, 556)],   # i2 @0 | o2 @64
    7: [(0, 44, 856)],                   # u2 @0
}
# Recurrent M-units (13 units over Wcat cols
# [i 0:300 | o 300:600 | u 600:900 | fL 900:1200 | fR 1200:1500]):
REC_SLOTS = {
    0: [(0, 128, 0)], 1: [(0, 128, 128)],
    2: [(0, 128, 300)], 3: [(0, 128, 428)],
    4: [(0, 128, 900)], 5: [(0, 128, 1028)],     # fL
    6: [(0, 128, 1200)], 7: [(0, 128, 1328)],    # fR
    8: [(0, 128, 600)], 9: [(0, 128, 728)],      # u
    10: [(0, 44, 256), (64, 108, 556)],          # i2 @0 | o2 @64
    11: [(0, 44, 856), (64, 108, 1156)],         # u2 @0 | fL2 @64
    12: [(0, 44, 1456)],                         # fR2 @0
}


# ---------------------------------------------------------------- host packing
def _pack_weights(Wfioux, b_fioux, Wiouh, Wfh):
    f4 = np.float32
    Wiou = np.asarray(Wfioux[:, 300:1200], f4)            # [300, 900]
    bleaf = np.asarray(b_fioux[300:1200], f4)             # [900]
    wleaf = np.zeros((3, 128, 8 * 128), f4)
    kch_l = [(0, 128), (128, 256), (256, 300)]
    for kc, (ra, rb) in enumerate(kch_l):
        for m, slots in LEAF_SLOTS.items():
            for (r0, r1, c0) in slots:
                wleaf[kc, 0: rb - ra, 128 * m + r0: 128 * m + r1] = \
                    Wiou[ra:rb, c0: c0 + (r1 - r0)]
    # bias via ones-row at partition 44 of K-chunk 2
    for m, slots in LEAF_SLOTS.items():
        for (r0, r1, c0) in slots:
            wleaf[2, 44, 128 * m + r0: 128 * m + r1] = bleaf[c0: c0 + (r1 - r0)]

    Wcat = np.concatenate([Wiouh, Wfh], axis=1).astype(f4)  # [600, 1500]
    bf = np.asarray(b_fioux[0:300], f4)
    bias_cat = np.concatenate(
        [b_fioux[300:600], b_fioux[600:900], b_fioux[900:1200], bf, bf]
    ).astype(f4)
    wrec = np.zeros((5, 128, 13 * 128), f4)
    # K-chunks: 0: hL[0:128], 1: hL[128:256], 2: packed hL[256:300]@0:44 +
    # ones@44 + hR[256:300]@64:108, 3: hR[0:128], 4: hR[128:256]
    kch_r = [(0, 128), (128, 256), None, (300, 428), (428, 556)]
    for kc, span in enumerate(kch_r):
        if span is None:
            continue
        ra, rb = span
        for m, slots in REC_SLOTS.items():
            for (r0, r1, c0) in slots:
                wrec[kc, 0: rb - ra, 128 * m + r0: 128 * m + r1] = \
                    Wcat[ra:rb, c0: c0 + (r1 - r0)]
    for m, slots in REC_SLOTS.items():
        for (r0, r1, c0) in slots:
            wrec[2, 0:44, 128 * m + r0: 128 * m + r1] = \
                Wcat[256:300, c0: c0 + (r1 - r0)]
            wrec[2, 44, 128 * m + r0: 128 * m + r1] = bias_cat[c0: c0 + (r1 - r0)]
            wrec[2, 64:108, 128 * m + r0: 128 * m + r1] = \
                Wcat[556:600, c0: c0 + (r1 - r0)]
    wleaf_f = np.ascontiguousarray(
        wleaf.transpose(1, 0, 2).reshape(128, 3 * 1024))
    wrec_f = np.ascontiguousarray(
        wrec.transpose(1, 0, 2).reshape(128, 5 * 1664))
    # fp8 copies of the main f-gate units, scaled by S_W.  Layout:
    # [0:1024] kc0/kc1 pairs (4 units x [two x 128]), [1024:2048] kc3/kc4,
    # [2048:2560] kc2 (4 x 128).
    import ml_dtypes
    w8 = np.zeros((128, 2560), np.float32)
    for fu, u in enumerate(F_UNITS):
        c0 = REC_SLOTS[u][0][2]
        for two, ra in ((0, 0), (1, 128)):
            w8[:, fu * 256 + two * 128: fu * 256 + two * 128 + 128] =                 Wcat[ra: ra + 128, c0: c0 + 128]
        for two, ra in ((0, 300), (1, 428)):
            w8[:, 1024 + fu * 256 + two * 128: 1024 + fu * 256 + two * 128 + 128] =                 Wcat[ra: ra + 128, c0: c0 + 128]
        w8[0:44, 2048 + fu * 128: 2048 + fu * 128 + 128] =             Wcat[256:300, c0: c0 + 128]
        w8[44, 2048 + fu * 128: 2048 + fu * 128 + 128] = bias_cat[c0: c0 + 128]
        w8[64:108, 2048 + fu * 128: 2048 + fu * 128 + 128] =             Wcat[556:600, c0: c0 + 128]
    wrec8 = (w8 * S_W).astype(ml_dtypes.float8_e4m3fn)
    return (wleaf_f.astype(np.float16), wrec_f.astype(np.float16), wrec8)


def _check_topology(left_idx, right_idx, leaf_mask):
    li = np.asarray(left_idx); ri = np.asarray(right_idx)
    prev = np.arange(N_LEAVES); nid = N_LEAVES
    ok = bool((np.asarray(leaf_mask)[:N_LEAVES] == 1).all())
    ok &= bool((np.asarray(leaf_mask)[N_LEAVES:] == 0).all())
    while len(prev) > 1:
        cur = []
        for k in range(0, len(prev), 2):
            ok &= bool(li[nid] == prev[k]) and bool(ri[nid] == prev[k + 1])
            cur.append(nid); nid += 1
        prev = np.asarray(cur)
    return ok


# ---------------------------------------------------------------- bass program
def _ev_od(ap, b=B_LOC):
    """Block-dense [p, X] (node-major, X = m*2*b) -> (even, odd) [p, m, b]."""
    r = ap.rearrange("p (m two b) -> p m two b", two=2, b=b)
    return r[:, :, 0, :], r[:, :, 1, :]


def _mb(ap, b=B_LOC):
    return ap.rearrange("p (m b) -> p m b", b=b)


def build_program():
    nc = bacc.Bacc("TRN2", target_bir_lowering=False, debug=False)

    xt_d = nc.dram_tensor("xt", [128, 2, XCOLS], F16, kind="ExternalInput").ap()
    x2t_d = nc.dram_tensor("x2t", [44, XCOLS], F16, kind="ExternalInput").ap()
    wleaf_d = nc.dram_tensor("wleaf", [128, 3 * 8 * 128], F16,
                             kind="ExternalInput").ap()
    wrec_d = nc.dram_tensor("wrec", [128, 5 * 13 * 128], F16,
                            kind="ExternalInput").ap()
    wrec8_d = nc.dram_tensor("wrec8", [128, 2560], F8,
                             kind="ExternalInput").ap()
    cons_d = nc.dram_tensor("cons", [84, 2 * LB], F16, kind="ExternalInput").ap()
    out_d = nc.dram_tensor("out", [128, 6 * B_LOC], F16,
                       kind="ExternalOutput").ap()

    with ExitStack() as ctx:
        tc = ctx.enter_context(tile.TileContext(nc))
        _build(ctx, tc, xt_d, x2t_d, wleaf_d, wrec_d, wrec8_d, cons_d, out_d)

    nc.compile()
    return nc


def _build(ctx, tc, xt_d, x2t_d, wleaf_d, wrec_d, wrec8_d, cons_d, out_d):
    nc = tc.nc

    wpool = ctx.enter_context(tc.tile_pool(name="wpool", bufs=1))
    state_pool = ctx.enter_context(tc.tile_pool(name="state", bufs=1))

    # ---- weights resident in SBUF (leaf weights first: needed immediately)
    wleaf_t = wpool.tile([128, 3 * 8 * 128], F16, name="wleaf")
    nc.sync.dma_start(wleaf_t[:], wleaf_d[:])
    wleaf_sb = [wleaf_t[:, k * 1024: (k + 1) * 1024] for k in range(3)]
    wrec_t = wpool.tile([128, 5 * 13 * 128], F16, name="wrec")
    wrec_sb = [wrec_t[:, k * 1664: (k + 1) * 1664] for k in range(5)]
    wrec8_t = wpool.tile([128, 2560], F8, name="wrec8")

    # ---- persistent SBUF state for levels 1..8
    # h01/c01 [128, 2R]: chunks 0,1, each [E | O].  h2p [128, R/2]: chunk-2
    # packed E@0:44 / O@64:108 (+ ones row 44 for the GEMM bias).
    # c2p [128, R/2]: chunk-2 E@64:108 / O@0:44 (so fL2@64*CL2 and fR2@0*CR2
    # have matching input base partitions).
    ST = {}
    for lvl in range(1, 9):
        R = R_LVL[lvl]
        h01 = state_pool.tile([128, 2 * R], F16, name=f"h01_{lvl}")
        h2p = state_pool.tile([128, R // 2], F16, name=f"h2p_{lvl}")
        c01 = state_pool.tile([128, 2 * R], F16, name=f"c01_{lvl}")
        c2p = state_pool.tile([128, R // 2], F16, name=f"c2p_{lvl}")
        ST[lvl] = dict(h01=h01, h2p=h2p, c01=c01, c2p=c2p, R=R)

    # persistent double-buffered leaf tiles that carry constant rows
    x2_t = [state_pool.tile([128, LB], F16, name="x2_0")]
    x2_t.append(x2_t[0])
    lh2p_t = []
    for i in range(2):
        lh2p_t.append(state_pool.tile([128, LB // 2], F16, name=f"lh2p_{i}"))

    def _pad_dmas():
        """Pad/ones constants for tiles not needed in the first block; emitted
        after the block-0 x DMAs so compute starts immediately."""
        for i in range(2):
            t = lh2p_t[i]
            nc.sync.dma_start(t[44:64, :], cons_d[0:20, : LB // 2])
            nc.sync.dma_start(t[108:128, :], cons_d[1:21, : LB // 2])
        for lvl in range(1, 9):
            R = R_LVL[lvl]
            h2p = ST[lvl]["h2p"]
            nc.sync.dma_start(h2p[44:64, :], cons_d[0:20, : R // 2])
            nc.sync.dma_start(h2p[108:128, :], cons_d[1:21, : R // 2])

    # ---- pools
    xpool = ctx.enter_context(tc.tile_pool(name="xpool", bufs=2))
    glpool = ctx.enter_context(tc.tile_pool(name="gl", bufs=3))
    lpool = ctx.enter_context(tc.tile_pool(name="lpool", bufs=2))
    gpool = ctx.enter_context(tc.tile_pool(name="g", bufs=2))
    pspool = ctx.enter_context(tc.tile_pool(name="ps", bufs=4, space="PSUM"))
    tmp1 = ctx.enter_context(tc.tile_pool(name="tmp1", bufs=1))
    tmp2 = ctx.enter_context(tc.tile_pool(name="tmp2", bufs=2))
    tmp3 = ctx.enter_context(tc.tile_pool(name="tmp3", bufs=1))
    opool = ctx.enter_context(tc.tile_pool(name="o", bufs=1))
    s8pool = ctx.enter_context(tc.tile_pool(name="s8", bufs=1))

    # ================================================================ helpers
    def leaf_gemm(xk, s, Gl):
        """Leaf gates for sub-chunk s (512 cols): 4 psum pairs, 5 ACTs."""
        n0 = s * NF
        for pi in range(4):
            ua, ub = 2 * pi, 2 * pi + 1
            ps = pspool.tile([128, 2 * NF], F32, tag="ps", name=f"psl{pi}")
            for j, u in enumerate((ua, ub)):
                rows = (128, 128, 128, 128, 128, 128, 108, 44)[u]
                off = j * NF
                for kc in range(3):
                    nc.tensor.matmul(
                        ps[0:rows, off: off + NF],
                        wleaf_sb[kc][:, 128 * u: 128 * u + rows],
                        xk[kc][:, n0: n0 + NF],
                        start=(kc == 0), stop=(kc == 2))
            if pi < 3:
                func = SIG if pi < 2 else TANH
                nc.scalar.activation(Gl[:, 2 * pi * NF: (2 * pi + 2) * NF],
                                     ps[:, :], func)
            else:
                nc.scalar.activation(Gl[0:108, 6 * NF: 7 * NF],
                                     ps[0:108, 0:NF], SIG)
                nc.scalar.activation(Gl[0:44, 7 * NF: 8 * NF],
                                     ps[0:44, NF: NF + NF], TANH)

    def rec_gemm(rhs_k, PB, G, fp8=True):
        """Recurrent gates for one block of PB cols.  The i/o/u units and the
        tail units run fp16 (5 K-chunk matmuls each); the 4 main f units run
        fp8 DoubleRow (2 DR pairs + 1 plain fp8 K-chunk) against an
        S_H-scaled fp8 copy of the child h staged on GpSimd.  The f-gate ACTs
        descale by 1/(S_W*S_H)."""
        # stage an S_H-scaled fp8 rhs copy (5 chunks side by side) on DVE
        if fp8:
            st8 = s8pool.tile([128, 5 * NF], F8, tag="st8", name="st8")
            for kc in range(5):
                nc.vector.tensor_scalar_mul(st8[:, kc * PB: (kc + 1) * PB],
                                            rhs_k[kc], S_H)
            dr = st8[:, 0: 5 * PB].rearrange("p (kc n) -> p kc n", kc=5)

        # iou + tail pairs (fp16), then f pairs (fp8)
        for pi, units in enumerate(((0, 1), (2, 3), (8, 9), (10, 11), (12,),
                                    (4, 5), (6, 7))):
            ps = pspool.tile([128, 2 * NF], F32, tag="ps", name=f"psr{pi}")
            for j, u in enumerate(units):
                off = j * PB
                if u in F_UNITS and fp8:
                    fu = u - 4
                    w01 = wrec8_t[:, fu * 256: (fu + 1) * 256].rearrange(
                        "p (two m) -> p two m", two=2)
                    w34 = wrec8_t[:, 1024 + fu * 256: 1024 + (fu + 1) * 256]                         .rearrange("p (two m) -> p two m", two=2)
                    w2 = wrec8_t[:, 2048 + fu * 128: 2048 + (fu + 1) * 128]
                    nc.tensor.matmul(ps[:, off: off + PB], w01, dr[:, 0:2, :],
                                     start=True, stop=False,
                                     perf_mode=mybir.MatmulPerfMode.DoubleRow)
                    nc.tensor.matmul(ps[:, off: off + PB], w34, dr[:, 3:5, :],
                                     start=False, stop=False,
                                     perf_mode=mybir.MatmulPerfMode.DoubleRow)
                    nc.tensor.matmul(ps[:, off: off + PB], w2,
                                     st8[:, 2 * PB: 3 * PB],
                                     start=False, stop=True)
                    continue
                rows = 108 if u in (10, 11) else (44 if u == 12 else 128)
                for kc in range(5):
                    nc.tensor.matmul(
                        ps[0:rows, off: off + PB],
                        wrec_sb[kc][:, 128 * u: 128 * u + rows],
                        rhs_k[kc],
                        start=(kc == 0), stop=(kc == 4))
            if pi < 2:
                nc.scalar.activation(G[:, 2 * pi * PB: (2 * pi + 2) * PB],
                                     ps[:, 0: 2 * PB], SIG)
            elif pi == 2:
                nc.scalar.activation(G[:, 8 * PB: 10 * PB], ps[:, 0: 2 * PB],
                                     TANH)
            elif pi == 3:
                # T10 = [i2@0 | o2@64] all sigmoid; T11 = [u2@0 | fL2@64]
                nc.scalar.activation(G[0:108, 10 * PB: 11 * PB],
                                     ps[0:108, 0:PB], SIG)
                nc.scalar.activation(G[0:44, 11 * PB: 12 * PB],
                                     ps[0:44, PB: 2 * PB], TANH)
                nc.scalar.activation(G[64:108, 11 * PB: 12 * PB],
                                     ps[64:108, PB: 2 * PB], SIG)
            elif pi == 4:
                nc.scalar.activation(G[0:44, 12 * PB: 13 * PB],
                                     ps[0:44, 0:PB], SIG)
            elif pi == 5:
                nc.scalar.activation(G[:, 4 * PB: 6 * PB], ps[:, 0: 2 * PB],
                                     SIG,
                                     scale=1.0 / (S_W * S_H) if fp8 else 1.0)
            else:
                nc.scalar.activation(G[:, 6 * PB: 8 * PB], ps[:, 0: 2 * PB],
                                     SIG,
                                     scale=1.0 / (S_W * S_H) if fp8 else 1.0)

    def st_sl(t, R, ch, eo, q0, w):
        off = ch * R + eo * (R // 2) + q0
        return t[:, off: off + w]

    def rec_ew(G, PB, CL, CR, dst, p0):
        """Elementwise for a recurrent block. G gates [128, 13*PB].
        CL/CR: (c0, c1, c2) child-c dense APs [., PB] (c2: 44 rows at base
        64 (CL) / 0 (CR) to match fL2/fR2 partition bases).
        dst: ST[lvl] dict, or ('root', oc01, oc2, oh01, oh2) for level 9."""
        N = PB
        gi = [G[:, 0:N], G[:, N: 2 * N], G[0:44, 10 * N: 11 * N]]
        go = [G[:, 2 * N: 3 * N], G[:, 3 * N: 4 * N], G[64:108, 10 * N: 11 * N]]
        gfL = [G[:, 4 * N: 5 * N], G[:, 5 * N: 6 * N], G[64:108, 11 * N: 12 * N]]
        gfR = [G[:, 6 * N: 7 * N], G[:, 7 * N: 8 * N], G[0:44, 12 * N: 13 * N]]
        gu = [G[:, 8 * N: 9 * N], G[:, 9 * N: 10 * N], G[0:44, 11 * N: 12 * N]]

        t1 = tmp1.tile([128, 2 * NF], F16, tag="t1", name="t1")
        t2 = tmp1.tile([128, 2 * NF], F16, tag="t2", name="t2")
        fc = tmp1.tile([128, 2 * NF], F16, tag="fc", name="fc")
        iu = tmp1.tile([128, 2 * NF], F16, tag="iu", name="iu")
        t1_2 = tmp1.tile([44, NF], F16, tag="t1_2", name="t1_2")
        t2_2 = tmp1.tile([44, NF], F16, tag="t2_2", name="t2_2")
        fc2 = tmp1.tile([44, NF], F16, tag="fc2", name="fc2")
        iu2 = tmp1.tile([44, NF], F16, tag="iu2", name="iu2")

        for ch in range(2):
            nc.vector.tensor_tensor(t1[:, ch * N: (ch + 1) * N], gfL[ch],
                                    CL[ch], MUL)
            nc.vector.tensor_tensor(t2[:, ch * N: (ch + 1) * N], gfR[ch],
                                    CR[ch], MUL)
        nc.vector.tensor_tensor(t1_2[:, :N], gfL[2], CL[2], MUL)
        nc.vector.tensor_tensor(t2_2[:, :N], gfR[2], CR[2], MUL)
        nc.vector.tensor_tensor(fc[:, : 2 * N], t1[:, : 2 * N], t2[:, : 2 * N],
                                ADD)
        nc.vector.tensor_tensor(fc2[:, :N], t1_2[:, :N], t2_2[:, :N], ADD)
        nc.vector.tensor_tensor(iu[:, : 2 * N], G[:, 0: 2 * N],
                                G[:, 8 * N: 10 * N], MUL)
        nc.vector.tensor_tensor(iu2[:, :N], gi[2], gu[2], MUL)

        if isinstance(dst, tuple) and dst[0] == "root":
            _, oc01, oc2, oh01, oh2 = dst
            nc.vector.tensor_tensor(oc01[:, : 2 * N], iu[:, : 2 * N],
                                    fc[:, : 2 * N], ADD)
            nc.vector.tensor_tensor(oc2[:, :N], iu2[:, :N], fc2[:, :N], ADD)
            th = tmp3.tile([128, 2 * NF], F16, tag="th", name="th")
            th2 = tmp3.tile([128, NF], F16, tag="th2", name="th2")
            nc.scalar.activation(th[:, : 2 * N], oc01[:, : 2 * N], TANH)
            nc.scalar.activation(th2[64:108, :N], oc2[:, :N], TANH)
            nc.vector.tensor_tensor(oh01[:, : 2 * N], G[:, 2 * N: 4 * N],
                                    th[:, : 2 * N], MUL)
            nc.vector.tensor_tensor(oh2[:, :N], go[2], th2[64:108, :N], MUL)
            return

        st = dst
        R = st["R"]
        q0, hw = p0 // 2, PB // 2
        # c into a dense block-local tile first: tanh + h writes come straight
        # off it (short critical path to the next level's GEMM); the
        # deinterleaved c-state copies trail after the h writes.
        cb, cb2 = t1, t2          # t1/t2 are dead after the fc add
        nc.vector.tensor_tensor(cb[:, : 2 * N], iu[:, : 2 * N], fc[:, : 2 * N],
                                ADD)
        nc.vector.tensor_tensor(cb2[64:108, :N], iu2[:, :N], fc2[:, :N], ADD)
        th = tmp3.tile([128, 2 * NF], F16, tag="th", name="th")
        th2 = tmp3.tile([128, NF], F16, tag="th2", name="th2")
        nc.scalar.activation(th[:, : 2 * N], cb[:, : 2 * N], TANH)
        nc.scalar.activation(th2[64:108, :N], cb2[64:108, :N], TANH)

        for ch in range(2):
            oe, oo = _ev_od(go[ch])
            the, tho_ = _ev_od(th[:, ch * N: (ch + 1) * N])
            nc.vector.tensor_tensor(_mb(st_sl(st["h01"], R, ch, 0, q0, hw)),
                                    oe, the, MUL)
            nc.vector.tensor_tensor(_mb(st_sl(st["h01"], R, ch, 1, q0, hw)),
                                    oo, tho_, MUL)
        oe, oo = _ev_od(go[2])
        the, tho_ = _ev_od(th2[64:108, :N])
        nc.vector.tensor_tensor(_mb(st["h2p"][0:44, q0: q0 + hw]), oe, the,
                                MUL)
        nc.vector.tensor_tensor(_mb(st["h2p"][64:108, q0: q0 + hw]), oo, tho_,
                                MUL)

        # trailing deinterleaved c-state copies (consumed by the NEXT level's
        # f*C multiplies, well after its GEMM)
        for ch in range(2):
            cbe, cbo = _ev_od(cb[:, ch * N: (ch + 1) * N])
            nc.vector.tensor_scalar_mul(
                _mb(st_sl(st["c01"], R, ch, 0, q0, hw)), cbe, 1.0)
            nc.vector.tensor_scalar_mul(
                _mb(st_sl(st["c01"], R, ch, 1, q0, hw)), cbo, 1.0)
        cbe, cbo = _ev_od(cb2[64:108, :N])
        nc.vector.tensor_scalar_mul(_mb(st["c2p"][64:108, q0: q0 + hw]), cbe,
                                    1.0)
        nc.vector.tensor_scalar_mul(_mb(st["c2p"][0:44, q0: q0 + hw]), cbo,
                                    1.0)

    def leaf_ew(Gl, s, lh01, lh2p, lc01, lc2p):
        """Leaf elementwise for sub-chunk s (512 cols): c = i*u, h = o*tanh(c).
        Writes deinterleaved into the LB-wide block-local leaf tiles."""
        N = NF
        q0, hw = s * (NF // 2), NF // 2
        gi = [Gl[:, 0:N], Gl[:, N: 2 * N], Gl[0:44, 6 * N: 7 * N]]
        go = [Gl[:, 2 * N: 3 * N], Gl[:, 3 * N: 4 * N], Gl[64:108, 6 * N: 7 * N]]
        gu = [Gl[:, 4 * N: 5 * N], Gl[:, 5 * N: 6 * N], Gl[0:44, 7 * N: 8 * N]]

        for ch in range(2):
            ie, io = _ev_od(gi[ch])
            ue, uo = _ev_od(gu[ch])
            nc.vector.tensor_tensor(_mb(st_sl(lc01, LB, ch, 0, q0, hw)), ie, ue,
                                    MUL)
            nc.vector.tensor_tensor(_mb(st_sl(lc01, LB, ch, 1, q0, hw)), io, uo,
                                    MUL)
        i2e, i2o = _ev_od(gi[2])
        u2e, u2o = _ev_od(gu[2])
        nc.vector.tensor_tensor(_mb(lc2p[64:108, q0: q0 + hw]), i2e, u2e, MUL)
        nc.vector.tensor_tensor(_mb(lc2p[0:44, q0: q0 + hw]), i2o, u2o, MUL)

        th = tmp2.tile([128, 2 * NF], F16, tag="thl", name="lth")
        th2 = tmp3.tile([128, NF], F16, tag="th2l", name="lth2")
        c4 = lc01.rearrange("p (ch eo q) -> p ch eo q", ch=2, eo=2)
        tho = th[:, : 2 * N].rearrange("p (ch eo q) -> p ch eo q", ch=2, eo=2)
        nc.scalar.activation(tho, c4[:, :, :, q0: q0 + hw], TANH)
        nc.scalar.activation(th2[64:108, 0:hw], lc2p[64:108, q0: q0 + hw],
                             TANH)
        nc.scalar.activation(th2[64:108, hw:N], lc2p[0:44, q0: q0 + hw], TANH)

        for ch in range(2):
            oe, oo = _ev_od(go[ch])
            nc.vector.tensor_tensor(_mb(st_sl(lh01, LB, ch, 0, q0, hw)), oe,
                                    _mb(th[:, ch * N: ch * N + hw]), MUL)
            nc.vector.tensor_tensor(
                _mb(st_sl(lh01, LB, ch, 1, q0, hw)), oo,
                _mb(th[:, ch * N + hw: ch * N + N]), MUL)
        oe, oo = _ev_od(go[2])
        nc.vector.tensor_tensor(_mb(lh2p[0:44, q0: q0 + hw]), oe,
                                _mb(th2[64:108, 0:hw]), MUL)
        nc.vector.tensor_tensor(_mb(lh2p[64:108, q0: q0 + hw]), oo,
                                _mb(th2[64:108, hw:N]), MUL)

    # ================================================================ phase A
    # leaves + level-1, software-pipelined: L1 GEMM of block b-1 is emitted
    # after the leaf GEMMs of block b so the PE never waits on leaf DVE.
    n_blk = XCOLS // LB                       # 8 blocks
    pend = None                               # (lh01, lh2p, lc01, lc2p, blk)

    def l1_block(lh01, lh2p, lc01, lc2p, blk):
        rhs_k = [st_sl(lh01, LB, 0, 0, 0, NF), st_sl(lh01, LB, 1, 0, 0, NF),
                 lh2p[:, :],
                 st_sl(lh01, LB, 0, 1, 0, NF), st_sl(lh01, LB, 1, 1, 0, NF)]
        G = gpool.tile([128, 13 * NF], F16, tag="G", name="G1")
        rec_gemm(rhs_k, NF, G)
        CL = [st_sl(lc01, LB, 0, 0, 0, NF), st_sl(lc01, LB, 1, 0, 0, NF),
              lc2p[64:108, :]]
        CR = [st_sl(lc01, LB, 0, 1, 0, NF), st_sl(lc01, LB, 1, 1, 0, NF),
              lc2p[0:44, :]]
        rec_ew(G, NF, CL, CR, ST[1], blk * NF)

    for blk in range(n_blk):
        c0 = blk * LB
        x01 = xpool.tile([128, 2 * LB], F16, tag="x01", name="x01")
        x2 = x2_t[blk % 2]
        nc.sync.dma_start(x01.rearrange("p (two b) -> p two b", two=2),
                          xt_d[:, :, c0: c0 + LB])
        nc.sync.dma_start(x2[0:44, :], x2t_d[:, c0: c0 + LB])
        if blk == 0:
            nc.sync.dma_start(x2[44:128, :], cons_d[0:84, :LB])
            nc.sync.dma_start(wrec_t[:], wrec_d[:])
            nc.sync.dma_start(wrec8_t[:], wrec8_d[:])
        elif blk == 1:
            _pad_dmas()
        xk = [x01[:, 0:LB], x01[:, LB: 2 * LB], x2]

        lh01 = lpool.tile([128, 2 * LB], F16, tag="lh01", name="lh01")
        lh2p = lh2p_t[blk % 2]
        lc01 = lpool.tile([128, 2 * LB], F16, tag="lc01", name="lc01")
        lc2p = lpool.tile([128, LB // 2], F16, tag="lc2p", name="lc2p")

        Gls = []
        for s in range(2):
            Gl = glpool.tile([128, 8 * NF], F16, tag="Gl", name="Gl")
            leaf_gemm(xk, s, Gl)
            Gls.append(Gl)
        if pend is not None:
            l1_block(*pend)
        for s in range(2):
            leaf_ew(Gls[s], s, lh01, lh2p, lc01, lc2p)
        pend = (lh01, lh2p, lc01, lc2p, blk)
    l1_block(*pend)

    # ================================================================ phase B
    for lvl in range(2, 10):
        R = R_LVL[lvl]
        Rp = R_LVL[lvl - 1]
        PB = min(NF, R)
        prev = ST[lvl - 1]
        for p0 in range(0, R, PB):
            rhs_k = [st_sl(prev["h01"], Rp, 0, 0, p0, PB),
                     st_sl(prev["h01"], Rp, 1, 0, p0, PB),
                     prev["h2p"][:, p0: p0 + PB],
                     st_sl(prev["h01"], Rp, 0, 1, p0, PB),
                     st_sl(prev["h01"], Rp, 1, 1, p0, PB)]
            G = gpool.tile([128, 13 * NF], F16, tag="G", name=f"G{lvl}")
            rec_gemm(rhs_k, PB, G[:, : 13 * PB], fp8=(PB == NF))
            CL = [st_sl(prev["c01"], Rp, 0, 0, p0, PB),
                  st_sl(prev["c01"], Rp, 1, 0, p0, PB),
                  prev["c2p"][64:108, p0: p0 + PB]]
            CR = [st_sl(prev["c01"], Rp, 0, 1, p0, PB),
                  st_sl(prev["c01"], Rp, 1, 1, p0, PB),
                  prev["c2p"][0:44, p0: p0 + PB]]
            if lvl < 9:
                rec_ew(G[:, : 13 * PB], PB, CL, CR, ST[lvl], p0)
            else:
                ot = opool.tile([128, 6 * B_LOC], F16, name="ot")
                nc.sync.dma_start(ot[44:128, 2 * B_LOC: 3 * B_LOC],
                                  cons_d[0:84, B_LOC: 2 * B_LOC])
                nc.sync.dma_start(ot[44:128, 5 * B_LOC: 6 * B_LOC],
                                  cons_d[0:84, B_LOC: 2 * B_LOC])
                rec_ew(G[:, : 13 * PB], PB, CL, CR,
                       ("root", ot[:, 0: 2 * B_LOC], ot[0:44, 2 * B_LOC: 3 * B_LOC],
                        ot[:, 3 * B_LOC: 5 * B_LOC], ot[0:44, 5 * B_LOC: 6 * B_LOC]),
                       p0)
                nc.sync.dma_start(out_d[:, :], ot[:, :])


# ---------------------------------------------------------------- runner
_CACHE = {}


def _get_program():
    if "nc" not in _CACHE:
        _CACHE["nc"] = build_program()
    return _CACHE["nc"]


def _host_inputs(inputs, Wfioux, b_fioux, Wiouh, Wfh):
    wleaf, wrec, wrec8 = _pack_weights(
        np.asarray(Wfioux, np.float32), np.asarray(b_fioux, np.float32),
        np.asarray(Wiouh, np.float32), np.asarray(Wfh, np.float32))
    cons = np.zeros((84, 2 * LB), np.float16)
    cons[0, :] = 1.0
    in_maps = []
    for core in range(N_CORES):
        x = np.asarray(inputs[core * B_LOC:(core + 1) * B_LOC, :N_LEAVES, :],
                       np.float32)
        xt_full = x.transpose(2, 1, 0).reshape(MEM, XCOLS).astype(np.float16)
        xt = np.ascontiguousarray(xt_full[0:256].reshape(2, 128, XCOLS)
                                  .transpose(1, 0, 2))
        x2t = np.ascontiguousarray(xt_full[256:300])
        in_maps.append({"xt": xt, "x2t": x2t, "wleaf": wleaf, "wrec": wrec,
                        "wrec8": wrec8, "cons": cons})
    return in_maps


def kernel(inputs, Wfioux, b_fioux, Wiouh, Wfh, left_idx, right_idx, leaf_mask,
           _trace=False, _trace_dir=None):
    inputs = np.asarray(inputs, np.float32)
    assert _check_topology(left_idx, right_idx, leaf_mask), \
        "tree topology does not match the expected complete binary tree"

    in_maps = _host_inputs(inputs, Wfioux, b_fioux, Wiouh, Wfh)
    nc = _get_program()
    res = run_bass_kernel_spmd(nc, in_maps, list(range(N_CORES)),
                               trace=_trace, tmpdir=_trace_dir)

    root_c = np.zeros((B, MEM), np.float32)
    root_h = np.zeros((B, MEM), np.float32)
    for core in range(N_CORES):
        out = np.asarray(res.results[core]["out"], np.float32)  # [128, 96]
        sl = slice(core * B_LOC, (core + 1) * B_LOC)
        root_c[sl, 0:128] = out[:, 0:16].T
        root_c[sl, 128:256] = out[:, 16:32].T
        root_c[sl, 256:300] = out[0:44, 32:48].T
        root_h[sl, 0:128] = out[:, 48:64].T
        root_h[sl, 128:256] = out[:, 64:80].T
        root_h[sl, 256:300] = out[0:44, 80:96].T
    _CACHE["last_results"] = res
    return root_c, root_h
